# revision 2
# baseline (speedup 1.0000x reference)
"""RBF kernel matrix on 8 Trainium2 NeuronCores.

out[i, j] = exp(-||x_i - y_j||^2),  x: (8192, 256) f32, y: (8192, 256) f32.

Strategy (per spec sharding hint): shard x row-wise across the 8 cores
(1024 rows each), replicate y; each core computes a (1024, 8192) tile.

Device-side math per output tile (m=128 partitions, n=512 free):
    psum   = matmul(lhsT=(-2x)^T chunk, rhs=y^T chunk) accumulated over
             the two 128-deep contraction chunks  -> -2 * x.y
    psum  += y2[n]        (DVE tensor_tensor add, y2 row replicated to
                           all 128 partitions once at startup)
    out    = exp(-psum - x2[m])   (ACT activation, scale=-1, per-partition
                                   bias = -x2)
         = exp(2 x.y - y2 - x2) = exp(-||x-y||^2)

Host-side prep (cheap numpy, analogous to sharding): transpose x/y into
d-major layout so the contraction dim lands on SBUF partitions, pre-scale
x by -2, precompute the squared norms.
"""

import numpy as np

M, N, D = 8192, 8192, 256
NCORES = 8
MLOC = M // NCORES          # 1024 rows of x per core
MT = MLOC // 128            # 8 m-tiles per core
GW = 2048                   # output column group width (one DMA slab)
NG = N // GW                # 4 column groups
NT = GW // 512              # 4 (128,512) tiles per slab

_CACHE = {}


def _build_nc():
    if "nc" in _CACHE:
        return _CACHE["nc"]

    import concourse.bacc as bacc
    import concourse.tile as tile
    import concourse.mybir as mybir

    f32 = mybir.dt.float32
    nc = bacc.Bacc(
        "TRN2",
        target_bir_lowering=False,
        debug=False,
        enable_asserts=False,
        num_devices=NCORES,
    )

    xt2 = nc.dram_tensor("xt2", [D, MLOC], f32, kind="ExternalInput").ap()
    yt = nc.dram_tensor("yt", [D, N], f32, kind="ExternalInput").ap()
    y2 = nc.dram_tensor("y2", [1, N], f32, kind="ExternalInput").ap()
    nx2 = nc.dram_tensor("nx2", [128, MT], f32, kind="ExternalInput").ap()
    out = nc.dram_tensor("out", [MLOC, N], f32, kind="ExternalOutput").ap()

    with tile.TileContext(nc) as tc:
        with (
            tc.tile_pool(name="persist", bufs=1) as persist,
            tc.tile_pool(name="slab", bufs=3) as slabs,
            tc.tile_pool(name="psum", bufs=8, space="PSUM") as psums,
        ):
            xt_sb = persist.tile([128, 2 * MLOC], f32, tag="xt")
            nc.sync.dma_start(xt_sb[:, 0:MLOC], xt2[0:128, :])
            nc.sync.dma_start(xt_sb[:, MLOC : 2 * MLOC], xt2[128:256, :])

            nx2_sb = persist.tile([128, MT], f32, tag="nx2")
            nc.sync.dma_start(nx2_sb[:], nx2[:])

            # replicate the y2 row to all 128 partitions by log2 doubling
            # (SBUF->SBUF DMAs; a step-0 partition broadcast is not allowed)
            y2b = persist.tile([128, N], f32, tag="y2b")
            nc.sync.dma_start(y2b[0:1, :], y2[:])
            k = 1
            while k < 128:
                nc.sync.dma_start(y2b[k : 2 * k, :], y2b[0:k, :])
                k *= 2

            yt_sb = {}
            for k in range(2):
                for g in range(NG):
                    t = persist.tile([128, GW], f32, tag=f"yt_{k}_{g}", name=f"yt_{k}_{g}")
                    nc.sync.dma_start(t[:], yt[k * 128 : (k + 1) * 128, g * GW : (g + 1) * GW])
                    yt_sb[(k, g)] = t

            for g in range(NG):
                for mt in range(MT):
                    slab = slabs.tile([128, GW], f32, tag="slab", name=f"slab_{g}_{mt}")
                    for ntl in range(NT):
                        nt = g * NT + ntl
                        ps = psums.tile([128, 512], f32, tag="ps", name=f"ps_{g}_{mt}_{ntl}")
                        nc.tensor.matmul(
                            ps[:],
                            xt_sb[:, mt * 128 : (mt + 1) * 128],
                            yt_sb[(0, g)][:, ntl * 512 : (ntl + 1) * 512],
                            start=True,
                            stop=False,
                        )
                        nc.tensor.matmul(
                            ps[:],
                            xt_sb[:, MLOC + mt * 128 : MLOC + (mt + 1) * 128],
                            yt_sb[(1, g)][:, ntl * 512 : (ntl + 1) * 512],
                            start=False,
                            stop=True,
                        )
                        nc.vector.tensor_tensor(
                            ps[:], ps[:], y2b[:, nt * 512 : (nt + 1) * 512],
                            op=mybir.AluOpType.add,
                        )
                        nc.scalar.activation(
                            slab[:, ntl * 512 : (ntl + 1) * 512],
                            ps[:],
                            mybir.ActivationFunctionType.Exp,
                            bias=nx2_sb[:, mt : mt + 1],
                            scale=-1.0,
                        )
                    nc.sync.dma_start(
                        out[mt * 128 : (mt + 1) * 128, g * GW : (g + 1) * GW], slab[:]
                    )

    nc.compile()
    _CACHE["nc"] = nc
    return nc


def _make_in_maps(x, y):
    x = np.ascontiguousarray(np.asarray(x, dtype=np.float32))
    y = np.ascontiguousarray(np.asarray(y, dtype=np.float32))
    yt = np.ascontiguousarray(y.T)                      # (256, 8192)
    y2 = np.sum(y * y, axis=1).reshape(1, N)            # (1, 8192)
    in_maps = []
    for c in range(NCORES):
        xs = x[c * MLOC : (c + 1) * MLOC]               # (1024, 256)
        xt2 = np.ascontiguousarray((-2.0 * xs).T)       # (256, 1024)
        nx2 = np.ascontiguousarray(
            (-np.sum(xs * xs, axis=1)).reshape(MT, 128).T  # (128, 8): [p, mt]
        )
        in_maps.append({"xt2": xt2, "yt": yt, "y2": y2, "nx2": nx2})
    return in_maps


def _run(x, y, trace=False, **kw):
    from concourse.bass_utils import run_bass_kernel_spmd

    nc = _build_nc()
    in_maps = _make_in_maps(x, y)
    res = run_bass_kernel_spmd(nc, in_maps, list(range(NCORES)), trace=trace, **kw)
    outp = np.concatenate([res.results[c]["out"] for c in range(NCORES)], axis=0)
    return outp, res


def kernel(x, y):
    return _run(x, y)[0]
